# revision 1
# baseline (speedup 1.0000x reference)
"""DiscreteKeyValueBottleneck Trainium2 kernel.

Reference computation (per batch b, codebook c, token t):
  idx = argmin_k ||batch[b,c,t,:] - keys[c,k,:]||^2
  mapped[b,c,t,:] = values[c, idx, :]
  pooled = mean_c mapped               -> [B, T, V]
  out = softmax(pooled, axis=T)        -> [B, T, V]

Sharding: one codebook per NeuronCore (expert-style, C == 8 == n_cores).
Each core computes its codebook's mapped values for ALL batches, a
ReduceScatter(add) over the batch axis combines codebooks and leaves each
core with 2 batches, on which it runs the softmax locally.

argmin is computed as argmax_k (x.k - |k|^2/2) via a 65-row contraction:
row 64 of the stationary operand is 1.0 and row 64 of the moving operand
is -|k|^2/2, so PSUM holds the final scores directly (fp32, error ~5e-6,
safely below the 6e-5 min top-2 gap of this input distribution).
"""

import numpy as np

B, C, T, D = 16, 8, 256, 64
K, V = 4096, 64
NCORES = 8
NT = B * T            # tokens per core (all batches, one codebook)
NTILES = NT // 128    # 32 token tiles
NCHUNK = K // 512     # 8 key chunks (one PSUM bank each)
BSH = B // NCORES     # batches per core after reduce-scatter

# offload part of the per-token max computation to the GPSIMD engine
# GPSIMD elementwise ops fail to compile on the bass2jax/axon execution
# path, so the max-tree offload stays disabled; DVE max8 is the verified path.
GPSIMD_MAX_TREE = False

_prog_cache = {}


def _build_program(single_core_sim=False):
    import concourse.bass as bass
    import concourse.tile as tile
    from concourse import bacc, mybir

    nc = bacc.Bacc('TRN2', target_bir_lowering=False, debug=False,
                   num_devices=1 if single_core_sim else NCORES)
    f32 = mybir.dt.float32

    xb = nc.dram_tensor('xb', [NT, D], f32, kind='ExternalInput').ap()
    keys = nc.dram_tensor('keys', [K, D], f32, kind='ExternalInput').ap()
    values = nc.dram_tensor('values', [K, V], f32, kind='ExternalInput').ap()
    ident_in = nc.dram_tensor('ident', [128, 128], f32, kind='ExternalInput').ap()
    out = nc.dram_tensor('out', [BSH * T, V], f32, kind='ExternalOutput').ap()

    # two half-size bounce buffers so the first ReduceScatter (batches 0-7)
    # can launch while the second half of the main loop is still running;
    # each core ends up with batches {i, i+8} (host reorders).
    partial_a = nc.dram_tensor('partial_a', [NT // 2, V], f32).ap()
    partial_b = nc.dram_tensor('partial_b', [NT // 2, V], f32).ap()
    rs_a = nc.dram_tensor('rs_a', [T, V], f32).ap()
    rs_b = nc.dram_tensor('rs_b', [T, V], f32).ap()

    with tile.TileContext(nc) as tc:
        with (
            tc.tile_pool(name='const', bufs=1) as constp,
            tc.tile_pool(name='loads', bufs=3) as loads,
            tc.tile_pool(name='scores', bufs=3) as scoresp,
            tc.tile_pool(name='small', bufs=2) as smallp,
            tc.tile_pool(name='tail', bufs=1) as tailp,
            tc.tile_pool(name='ptr', bufs=2, space='PSUM') as ptr,
            tc.tile_pool(name='pmain', bufs=4, space='PSUM') as pmain,
        ):
            ident = constp.tile([128, 128], f32)
            nc.sync.dma_start(ident[:], ident_in[:])

            # ---- keys prep (per chunk, so main MMs can start early) ----
            # keysc[j][0:64] = keys^T chunk j, row 64 = -|k|^2/2
            ones64 = constp.tile([64, 1], f32)
            nc.vector.memset(ones64[:], 1.0)
            keysc = []
            for j in range(NCHUNK):
                kc = constp.tile([65, 512], f32, tag=f'keysc{j}')
                keysc.append(kc)
                kld = loads.tile([128, 4 * D], f32, tag='kld')
                for q in range(4):
                    nc.sync.dma_start(
                        kld[:, q * D:(q + 1) * D],
                        keys[j * 512 + q * 128: j * 512 + (q + 1) * 128, :])
                pt = ptr.tile([64, 512], f32, tag='tr')
                for q in range(4):
                    nc.tensor.transpose(
                        pt[:, q * 128:(q + 1) * 128], kld[:, q * D:(q + 1) * D],
                        ident[:])
                nc.scalar.copy(kc[0:64, :], pt[:])
                # square and k2-evict run on the DVE (idle during the head);
                # keeping them off ACT shortens the serial keys-prep chain
                # that gates the first main matmul.
                sqj = loads.tile([64, 512], f32, tag='sqj')
                nc.vector.tensor_mul(sqj[:], kc[0:64, :], kc[0:64, :])
                pk2 = ptr.tile([1, 512], f32, tag='k2')
                nc.tensor.matmul(pk2[:], ones64[:], sqj[:],
                                 start=True, stop=True)
                nc.vector.tensor_scalar_mul(kc[64:65, :], pk2[:], -0.5)

            # ---- X prep (per token tile): xt[0:64] = X_k^T, row 64 = 1 ----
            xts = []
            for k in range(NTILES):
                xt = constp.tile([65, 128], f32, tag=f'xt{k}')
                xts.append(xt)
                nc.vector.memset(xt[64:65, :], 1.0)
                xld = loads.tile([128, D], f32, tag='xld')
                nc.sync.dma_start(xld[:], xb[k * 128:(k + 1) * 128, :])
                ptx = ptr.tile([64, 128], f32, tag='tr')
                nc.tensor.transpose(ptx[:], xld[:], ident[:])
                nc.scalar.copy(xt[0:64, :], ptx[:])

            # ---- main: scores + argmax + values-gather per token tile ----
            for k in range(NTILES):
                scores = scoresp.tile([128, K], f32, tag='scores')
                for j in range(NCHUNK):
                    pm = pmain.tile([128, 512], f32, tag='mm')
                    nc.tensor.matmul(
                        pm[:], xts[k][:], keysc[j][:],
                        start=True, stop=True)
                    nc.scalar.copy(scores[:, j * 512:(j + 1) * 512], pm[:])
                if GPSIMD_MAX_TREE:
                    # elementwise max-tree over the 8 chunks, mostly on the
                    # otherwise-idle GPSIMD engine; DVE only reduces the last
                    # 512-wide slab and runs max_index.
                    t4 = smallp.tile([128, 4 * 512], f32, tag='t4')
                    for h in range(4):
                        nc.gpsimd.tensor_max(
                            t4[:, h * 512:(h + 1) * 512],
                            scores[:, (2 * h) * 512:(2 * h + 1) * 512],
                            scores[:, (2 * h + 1) * 512:(2 * h + 2) * 512])
                    t2 = smallp.tile([128, 2 * 512], f32, tag='t2')
                    nc.gpsimd.tensor_max(t2[:, 0:512], t4[:, 0:512],
                                         t4[:, 512:1024])
                    nc.vector.tensor_max(t2[:, 512:1024], t4[:, 1024:1536],
                                         t4[:, 1536:2048])
                    t1 = smallp.tile([128, 512], f32, tag='t1')
                    nc.vector.tensor_max(t1[:], t2[:, 0:512], t2[:, 512:1024])
                    g = smallp.tile([128, 1], f32, tag='g')
                    nc.vector.tensor_reduce(g[:], t1[:],
                                            op=mybir.AluOpType.max,
                                            axis=mybir.AxisListType.X)
                    idx8 = smallp.tile([128, 8], mybir.dt.uint32, tag='idx8')
                    nc.vector.max_index(
                        idx8[:], g[:, 0:1].to_broadcast([128, 8]), scores[:])
                else:
                    mx8 = smallp.tile([128, 8], f32, tag='mx8')
                    nc.vector.max(mx8[:], scores[:])
                    idx8 = smallp.tile([128, 8], mybir.dt.uint32, tag='idx8')
                    nc.vector.max_index(idx8[:], mx8[:], scores[:])
                mapped = smallp.tile([128, V], f32, tag='mapped')
                nc.gpsimd.indirect_dma_start(
                    out=mapped[:], out_offset=None, in_=values[:],
                    in_offset=bass.IndirectOffsetOnAxis(ap=idx8[:, 0:1], axis=0))
                if k < NTILES // 2:
                    nc.sync.dma_start(partial_a[k * 128:(k + 1) * 128, :],
                                      mapped[:])
                else:
                    kk = k - NTILES // 2
                    nc.sync.dma_start(partial_b[kk * 128:(kk + 1) * 128, :],
                                      mapped[:])
                if not single_core_sim and k == NTILES // 2 - 1:
                    nc.gpsimd.collective_compute(
                        'ReduceScatter', mybir.AluOpType.add,
                        replica_groups=[list(range(NCORES))],
                        ins=[partial_a[:]], outs=[rs_a[:]])

            # ---- combine codebooks: second-half ReduceScatter ----
            if single_core_sim:
                # TimelineSim can't simulate collectives; stand in same-size
                # local copies so the tail still gets modeled.
                cp = tailp.tile([128, BSH * T // 128 * V], f32, tag='rscopy')
                for q in range(2):
                    nc.sync.dma_start(cp[:, q * V:(q + 1) * V],
                                      partial_a[q * 128:(q + 1) * 128, :])
                    nc.sync.dma_start(cp[:, (q + 2) * V:(q + 3) * V],
                                      partial_b[q * 128:(q + 1) * 128, :])
                for q in range(2):
                    nc.sync.dma_start(rs_a[q * 128:(q + 1) * 128, :],
                                      cp[:, q * V:(q + 1) * V])
                    nc.sync.dma_start(rs_b[q * 128:(q + 1) * 128, :],
                                      cp[:, (q + 2) * V:(q + 3) * V])
            else:
                nc.gpsimd.collective_compute(
                    'ReduceScatter', mybir.AluOpType.add,
                    replica_groups=[list(range(NCORES))],
                    ins=[partial_b[:]], outs=[rs_b[:]])

            # ---- softmax over T per (batch, v) on the local 2-batch shard --
            pts = ptr.tile([64, BSH * T], f32, tag='tr')
            for q in range(BSH * T // 128):
                sld = loads.tile([128, V], f32, tag='sld')
                rs_src = rs_a if q < 2 else rs_b
                nc.sync.dma_start(sld[:],
                                  rs_src[(q % 2) * 128:(q % 2 + 1) * 128, :])
                nc.tensor.transpose(pts[:, q * 128:(q + 1) * 128], sld[:],
                                    ident[:])
            sm = tailp.tile([64, BSH * T], f32)
            den = smallp.tile([64, BSH], f32, tag='den')
            for b in range(BSH):
                nc.scalar.activation(
                    sm[:, b * T:(b + 1) * T], pts[:, b * T:(b + 1) * T],
                    mybir.ActivationFunctionType.Exp,
                    scale=1.0 / C, accum_out=den[:, b:b + 1])
            rden = smallp.tile([64, BSH], f32, tag='rden')
            nc.vector.reciprocal(rden[:], den[:])
            for b in range(BSH):
                nc.vector.tensor_scalar(
                    out=sm[:, b * T:(b + 1) * T], in0=sm[:, b * T:(b + 1) * T],
                    scalar1=rden[:, b:b + 1], scalar2=None,
                    op0=mybir.AluOpType.mult)
            pso = ptr.tile([128, BSH * T // 128 * V], f32, tag='tr')
            so = tailp.tile([128, BSH * T // 128 * V], f32)
            for q in range(BSH * T // 128):
                nc.tensor.transpose(pso[:, q * V:(q + 1) * V],
                                    sm[:, q * 128:(q + 1) * 128],
                                    ident[0:64, 0:64])
            nc.scalar.copy(so[:], pso[:])
            for q in range(BSH * T // 128):
                nc.sync.dma_start(out[q * 128:(q + 1) * 128, :],
                                  so[:, q * V:(q + 1) * V])

    nc.compile()
    return nc


def _get_program():
    if 'nc' not in _prog_cache:
        _prog_cache['nc'] = _build_program()
    return _prog_cache['nc']


def kernel(batch, keys, values):
    from concourse import bass_utils

    nc = _get_program()
    ident = np.eye(128, dtype=np.float32)
    in_maps = []
    for c in range(NCORES):
        in_maps.append({
            'xb': np.ascontiguousarray(
                batch[:, c].reshape(NT, D).astype(np.float32)),
            'keys': np.ascontiguousarray(keys[c].astype(np.float32)),
            'values': np.ascontiguousarray(values[c].astype(np.float32)),
            'ident': ident,
        })
    res = bass_utils.run_bass_kernel_spmd(nc, in_maps,
                                          core_ids=list(range(NCORES)))
    # core i holds batches {i, i + 8} (split reduce-scatter halves)
    out = np.empty((B, T, V), dtype=np.float32)
    for i in range(NCORES):
        shard = res.results[i]['out'].reshape(BSH, T, V)
        out[i] = shard[0]
        out[i + NCORES] = shard[1]
    return out

